# revision 1
# baseline (speedup 1.0000x reference)
"""Trainium2 Bass kernel for nn_DTN_47459388620856 (grouped-moment2 norm +
2x2 pooled positional-attention renormalization).

v4 — engine-balanced:
  * partition = pooled patch j (2 tiles of 98/batch), free = (q, c).
  * stats: x*x TT + multi-chunk reduce; S = raw ACT Rsqrt; r = S^2 on DVE.
  * xn: ONE TT with inner-dim stride-0 broadcast of S (HW-probed).
  * positional einsum patch-major bf16; the A-side mix weight (1-mw)/4 is
    folded into its own stationary copy so the pooled sum feeds the PE raw.
  * Abq (= (1-mw)m_r + mw*mean_ln) is built straight from PSUM; the big
    num subtract runs on GPSIMD; iv = raw ACT Rsqrt(V0+bias); out=num*iv.
  * ACT calls are phase-grouped so the activation table swaps only twice
    per batch (Rsqrt group <-> Square group).
"""

import numpy as np


def _ensure_path():
    try:
        import concourse  # noqa: F401
    except ImportError:
        import sys
        for p in ("/opt/trn_rl_repo",):
            if p not in sys.path:
                sys.path.insert(0, p)


EPS = 1e-5
HEADS, RES, PS = 4, 28, 14
T, C = RES * RES, 768
CH = C // HEADS
P = PS * PS
JT = 98
NCORES = 8
BP = 4

_PROGRAM_CACHE = {}


def _sigmoid(v):
    return 1.0 / (1.0 + np.exp(-v.astype(np.float64)))


def _host_consts(mean_norm_weight, var_norm_weight, pos_w, pos_b):
    import ml_dtypes
    mw = _sigmoid(mean_norm_weight)
    vw = _sigmoid(var_norm_weight)

    ind = np.arange(PS)[None, :] - np.arange(PS)[:, None]
    indx = np.tile(ind, (PS, PS))
    indy = np.repeat(np.repeat(ind, PS, axis=0), PS, axis=1)
    rel = np.stack([indx, indy, indx * indx + indy * indy], -1).astype(np.float32)
    scores = rel @ pos_w.T.astype(np.float32) + pos_b.astype(np.float32)
    e = np.exp(scores - scores.max(axis=0, keepdims=True))
    pos = e / e.sum(axis=0, keepdims=True)
    pos_h = np.transpose(pos, (2, 0, 1)).astype(np.float64)   # (H, i, j)

    # posA folds (1-mw)/4 so A = posA @ xp_sum = (1-mw)*mean_r;
    # posB is unscaled: B = posB @ (sB*xp_sum)^2 = (1-vw)*mean2_r.
    sA = ((1.0 - mw) / 4.0)
    posA = np.zeros((2, JT, HEADS, 2, JT), np.float32)
    posB = np.zeros((2, JT, HEADS, 2, JT), np.float32)
    for ic in range(2):
        for jc in range(2):
            blk = np.transpose(
                pos_h[:, ic * JT:(ic + 1) * JT, jc * JT:(jc + 1) * JT],
                (1, 0, 2))
            posB[ic, :, :, jc, :] = blk
            posA[ic, :, :, jc, :] = blk * sA[None, :, None]
    posA_bf = posA.astype(ml_dtypes.bfloat16)
    posB_bf = posB.astype(ml_dtypes.bfloat16)

    sB = (np.sqrt(1.0 - vw) / 4.0).astype(np.float32)
    sG = (np.sqrt(1.0 - vw) / (1.0 - mw)).astype(np.float32)
    return posA_bf, posB_bf, sB, sG, mw.astype(np.float32), vw.astype(np.float32)


def _raw_act(eng, out, in_, func, mybir, bias=0.0, scale=1.0):
    ins = [eng.lower_ap(in_)]
    ins.append(eng.lower_ap(bias) if not isinstance(bias, float)
               else mybir.ImmediateValue(dtype=mybir.dt.float32, value=bias))
    ins.append(mybir.ImmediateValue(dtype=mybir.dt.float32, value=scale))
    ins.append(mybir.ImmediateValue(dtype=mybir.dt.float32, value=0.0))
    return eng.add_instruction(
        mybir.InstActivation(
            name=eng.bass.get_next_instruction_name(),
            func=func, ins=ins, outs=[eng.lower_ap(out)]))


def _build_program(consts):
    _ensure_path()
    from contextlib import ExitStack
    import concourse.bass as bass  # noqa: F401
    import concourse.tile as tile
    from concourse import bacc, mybir

    posA_bf, posB_bf, sB, sG, mw, vw = consts
    eqh = bool(np.all(mw == mw[0]) and np.all(vw == vw[0]))

    dt = mybir.dt.float32
    bt = mybir.dt.bfloat16
    AO = mybir.AluOpType
    AF = mybir.ActivationFunctionType
    AX = mybir.AxisListType

    nc = bacc.Bacc("TRN2", target_bir_lowering=False, debug=False,
                   enable_asserts=False)

    x_d = nc.dram_tensor("x", (BP, 14, 2, 14, 2, C), dt,
                         kind="ExternalInput").ap()
    pA_d = nc.dram_tensor("posA", (2, JT, HEADS, 2, JT), bt,
                          kind="ExternalInput").ap()
    pB_d = nc.dram_tensor("posB", (2, JT, HEADS, 2, JT), bt,
                          kind="ExternalInput").ap()
    out_d = nc.dram_tensor("out", (BP, 14, 2, 14, 2, C), dt,
                           kind="ExternalOutput").ap()

    x_re = x_d.transpose([0, 1, 3, 2, 4, 5])
    o_re = out_d.transpose([0, 1, 3, 2, 4, 5])

    with ExitStack() as ctx:
        tc = ctx.enter_context(tile.TileContext(nc))
        cpool = ctx.enter_context(tc.tile_pool(name="consts", bufs=1))
        xtp = ctx.enter_context(tc.tile_pool(name="xt", bufs=2))
        xnp = ctx.enter_context(tc.tile_pool(name="xn", bufs=2))
        xqp = ctx.enter_context(tc.tile_pool(name="xsq", bufs=2))
        smp = ctx.enter_context(tc.tile_pool(name="smalls", bufs=2))
        plp = ctx.enter_context(tc.tile_pool(name="pool", bufs=2))
        abp = ctx.enter_context(tc.tile_pool(name="ab", bufs=2))
        nmp = ctx.enter_context(tc.tile_pool(name="num", bufs=2))
        obp = ctx.enter_context(tc.tile_pool(name="outsb", bufs=2))
        ppp = ctx.enter_context(tc.tile_pool(name="ppsum", bufs=2,
                                             space="PSUM"))

        pA_sb, pB_sb = [], []
        for ic in range(2):
            tA = cpool.tile([JT, HEADS, 2, JT], bt, tag=f"posA{ic}")
            nc.sync.dma_start(tA[:], pA_d[ic])
            pA_sb.append(tA)
            tB = cpool.tile([JT, HEADS, 2, JT], bt, tag=f"posB{ic}")
            nc.sync.dma_start(tB[:], pB_d[ic])
            pB_sb.append(tB)

        for b in range(BP):
            xts, m2s = [], []
            # phase 1: load + m2 (sum of squares)
            for jc in range(2):
                xt = xtp.tile([JT, 2, 2, C], dt, tag="xt")
                for d in range(2):
                    nc.sync.dma_start(xt[:, d],
                                      x_re[b, jc * 7:(jc + 1) * 7, :, d])
                xts.append(xt)
                xflat = xt[:].rearrange("p d s c -> p (d s c)")
                xsq = xqp.tile([JT, 16, CH], bt, tag="xsq")
                nc.vector.tensor_mul(
                    xsq[:].rearrange("p a c -> p (a c)"), xflat, xflat)
                m2 = smp.tile([JT, 16], dt, tag="m2")
                nc.vector.reduce_sum(m2[:], xsq[:], axis=AX.X)
                m2s.append(m2)

            # phase 2: S (ACT Rsqrt, grouped)
            Ss = []
            for jc in range(2):
                S = smp.tile([JT, 16], dt, tag="S")
                _raw_act(nc.scalar, S[:], m2s[jc][:], AF.Rsqrt, mybir,
                         bias=EPS, scale=1.0 / CH)
                Ss.append(S)

            # phase 3: xn, per-token smalls, pooling
            xns, xpss, mmls, svls = [], [], [], []
            for jc in range(2):
                xt, m2, S = xts[jc], m2s[jc], Ss[jc]
                xseg = xt[:].rearrange("p d s (h c) -> p (d s h) c", h=HEADS)
                xn = xnp.tile([JT, 4, C], bt, tag="xn")
                nc.vector.tensor_mul(
                    xn[:].rearrange("p q (h c) -> p (q h) c", h=HEADS),
                    xseg, S[:].unsqueeze(2).broadcast_to([JT, 16, CH]))
                xns.append(xn)
                sxn = smp.tile([JT, 4], dt, tag="sxn")
                nc.vector.reduce_sum(sxn[:], xn[:], axis=AX.X)
                r_ = smp.tile([JT, 16], dt, tag="r")
                nc.vector.tensor_mul(r_[:], S[:], S[:])
                u = smp.tile([JT, 16], dt, tag="u")
                nc.vector.tensor_mul(u[:], m2[:], r_[:])
                su = smp.tile([JT, 4], dt, tag="su")
                nc.vector.reduce_sum(
                    su[:], u[:].rearrange("p (q h) -> p q h", q=4), axis=AX.X)
                mml = smp.tile([JT, 4], dt, tag="mml")
                nc.vector.tensor_scalar_mul(mml[:], sxn[:], float(mw[0]) / C)
                T1v = smp.tile([JT, 4], dt, tag="T1v")
                nc.vector.scalar_tensor_tensor(
                    out=T1v[:], in0=mml[:],
                    scalar=float(-vw[0] * C / (C - 1.0) / (mw[0] * mw[0])),
                    in1=mml[:], op0=AO.mult, op1=AO.mult)
                svla = smp.tile([JT, 4], dt, tag="svla")
                nc.vector.tensor_scalar(
                    out=svla[:], in0=su[:],
                    scalar1=float(vw[0] / (C - 1.0)), scalar2=EPS,
                    op0=AO.mult, op1=AO.add)
                svl = smp.tile([JT, 4], dt, tag="svl")
                nc.vector.tensor_add(svl[:], svla[:], T1v[:])
                mmls.append(mml)
                svls.append(svl)

                tmp2 = plp.tile([JT, 2, C], bt, tag="tmp2")
                nc.gpsimd.tensor_add(tmp2[:], xn[:, 0:2, :], xn[:, 2:4, :])
                xps = plp.tile([JT, C], bt, tag="xps")
                nc.vector.tensor_add(xps[:], tmp2[:, 0, :], tmp2[:, 1, :])
                xpss.append(xps)

            # phase 4: xpsq = (sB*xps)^2 (ACT Square, grouped)
            xpqs = []
            for jc in range(2):
                xpq = plp.tile([JT, C], bt, tag="xpq")
                if eqh:
                    nc.scalar.activation(xpq[:], xpss[jc][:], AF.Square,
                                         scale=float(sB[0]))
                else:
                    for h in range(HEADS):
                        hs = slice(h * CH, (h + 1) * CH)
                        nc.scalar.activation(xpq[:, hs], xpss[jc][:, hs],
                                             AF.Square, scale=float(sB[h]))
                xpqs.append(xpq)

            # phase 5: positional matmuls (A and B stationaries)
            pts = []
            for jc in range(2):
                pt = ppp.tile([JT, HEADS, 512], dt, tag="pt")
                for h in range(HEADS):
                    hs = slice(h * CH, (h + 1) * CH)
                    for ic in range(2):
                        nc.tensor.matmul(pt[:, h, 0:CH],
                                         pA_sb[ic][:, h, jc, :],
                                         xpss[ic][:, hs],
                                         start=(ic == 0), stop=(ic == 1))
                    for ic in range(2):
                        nc.tensor.matmul(pt[:, h, CH:2 * CH],
                                         pB_sb[ic][:, h, jc, :],
                                         xpqs[ic][:, hs],
                                         start=(ic == 0), stop=(ic == 1))
                pts.append(pt)

            # phase 6: SqA (ACT Square, grouped) + V0 + Abq from PSUM
            V0s, Abqs = [], []
            for jc in range(2):
                pt = pts[jc]
                sqa = abp.tile([JT, HEADS, CH], bt, tag="sqa")
                if eqh:
                    nc.scalar.activation(sqa[:], pt[:, :, 0:CH], AF.Square,
                                         scale=float(sG[0]))
                else:
                    for h in range(HEADS):
                        nc.scalar.activation(sqa[:, h], pt[:, h, 0:CH],
                                             AF.Square, scale=float(sG[h]))
                V0 = abp.tile([JT, C], bt, tag="V0")
                nc.vector.tensor_sub(
                    V0[:].rearrange("p (h c) -> p h c", h=HEADS),
                    pt[:, :, CH:2 * CH], sqa[:])
                V0s.append(V0)
                Abq = nmp.tile([JT, 4, C], bt, tag="Abq")
                for q in range(4):
                    nc.vector.tensor_scalar_add(
                        Abq[:, q, :].rearrange("p (h c) -> p h c", h=HEADS),
                        pt[:, :, 0:CH], mmls[jc][:, q:q + 1])
                Abqs.append(Abq)

            # phase 7: iv (ACT Rsqrt, grouped)
            ivs = []
            for jc in range(2):
                iv = nmp.tile([JT, 4, C], bt, tag="iv")
                for q in range(4):
                    _raw_act(nc.scalar, iv[:, q, :], V0s[jc][:], AF.Rsqrt,
                             mybir, bias=svls[jc][:, q:q + 1])
                ivs.append(iv)

            # phase 8: num (GPSIMD) + out (DVE) + DMA
            for jc in range(2):
                num = nmp.tile([JT, 4, C], bt, tag="num")
                nc.gpsimd.tensor_sub(num[:], xns[jc][:], Abqs[jc][:])
                outsb = obp.tile([JT, 4, C], dt, tag="outsb")
                nc.vector.tensor_mul(
                    outsb[:].rearrange("p q c -> p (q c)"),
                    num[:].rearrange("p q c -> p (q c)"),
                    ivs[jc][:].rearrange("p q c -> p (q c)"))
                osb4 = outsb[:].rearrange("p (d s) c -> p d s c", d=2, s=2)
                for d in range(2):
                    nc.scalar.dma_start(o_re[b, jc * 7:(jc + 1) * 7, :, d],
                                        osb4[:, d])

    nc.compile()
    return nc


def _make_in_maps(inputs):
    x = np.ascontiguousarray(np.asarray(inputs["x"], dtype=np.float32))
    cs = _host_consts(
        np.asarray(inputs["mean_norm_weight"], dtype=np.float32),
        np.asarray(inputs["var_norm_weight"], dtype=np.float32),
        np.asarray(inputs["pos_w"], dtype=np.float32),
        np.asarray(inputs["pos_b"], dtype=np.float32))
    posA_bf, posB_bf = cs[0], cs[1]
    in_maps = []
    for c in range(NCORES):
        m = {"posA": posA_bf, "posB": posB_bf,
             "x": np.ascontiguousarray(
                 x[c * BP:(c + 1) * BP]).reshape(BP, 14, 2, 14, 2, C)}
        in_maps.append(m)
    return in_maps


def kernel(x, weight, bias, mean_norm_weight, var_norm_weight, pos_w, pos_b):
    _ensure_path()
    from concourse import bass_utils

    x = np.asarray(x, dtype=np.float32)
    B = x.shape[0]
    weight = np.asarray(weight, dtype=np.float32)
    bias = np.asarray(bias, dtype=np.float32)

    consts = _host_consts(
        np.asarray(mean_norm_weight, dtype=np.float32),
        np.asarray(var_norm_weight, dtype=np.float32),
        np.asarray(pos_w, dtype=np.float32),
        np.asarray(pos_b, dtype=np.float32))

    key = "v4"
    if key not in _PROGRAM_CACHE:
        _PROGRAM_CACHE[key] = _build_program(consts)
    nc = _PROGRAM_CACHE[key]

    in_maps = _make_in_maps(dict(
        x=x, mean_norm_weight=mean_norm_weight,
        var_norm_weight=var_norm_weight, pos_w=pos_w, pos_b=pos_b))

    res = bass_utils.run_bass_kernel_spmd(nc, in_maps,
                                          core_ids=list(range(NCORES)))
    out = np.concatenate(
        [res.results[c]["out"].reshape(BP, T, C) for c in range(NCORES)],
        axis=0)
    assert out.shape == (B, T, C)
    out = out.astype(np.float32)

    if np.any(weight != 1.0):
        out = out * weight.reshape(1, 1, C)
    if np.any(bias != 0.0):
        out = out + bias.reshape(1, 1, C)
    return out



# revision 3
# speedup vs baseline: 1.4232x; 1.4232x over previous
"""Trainium2 Bass kernel for nn_DTN_47459388620856 (grouped-moment2 norm +
2x2 pooled positional-attention renormalization).

v5 — bf16 end-to-end + engine rebalance (HW-probed op rates):
  * x cast to bf16 on host; output returned bf16, upcast on host. Halves
    DMA traffic and makes every big DVE op eligible for the 2-byte 2x mode
    (probed: all-bf16 unit-stride tensor_tensor = 2x; broadcast/mixed = 1x;
    tensor_scalar with fp32 scalar-AP = 2x; reduce = 1x always).
  * DVE keeps only 2x-eligible big ops: xsq, halve-adds, reduces (halved
    first), pooling, num, out. ~23us/batch.
  * xn (S-broadcast mul, 1x anywhere) and V0 go to GpSimd.
  * ACT: S rsqrt, squares, Abq via Identity+bias-AP (no table load),
    iv via raw Rsqrt+bias-AP. Grouped: 2 table swaps/batch.
  * PE pos matmuls unchanged from v4.
"""

import numpy as np


def _ensure_path():
    try:
        import concourse  # noqa: F401
    except ImportError:
        import sys
        for p in ("/opt/trn_rl_repo",):
            if p not in sys.path:
                sys.path.insert(0, p)


EPS = 1e-5
HEADS, RES, PS = 4, 28, 14
T, C = RES * RES, 768
CH = C // HEADS
P = PS * PS
JT = 98
NCORES = 8
BP = 4

_PROGRAM_CACHE = {}


def _sigmoid(v):
    return 1.0 / (1.0 + np.exp(-v.astype(np.float64)))


def _host_consts(mean_norm_weight, var_norm_weight, pos_w, pos_b):
    import ml_dtypes
    mw = _sigmoid(mean_norm_weight)
    vw = _sigmoid(var_norm_weight)

    ind = np.arange(PS)[None, :] - np.arange(PS)[:, None]
    indx = np.tile(ind, (PS, PS))
    indy = np.repeat(np.repeat(ind, PS, axis=0), PS, axis=1)
    rel = np.stack([indx, indy, indx * indx + indy * indy], -1).astype(np.float32)
    scores = rel @ pos_w.T.astype(np.float32) + pos_b.astype(np.float32)
    e = np.exp(scores - scores.max(axis=0, keepdims=True))
    pos = e / e.sum(axis=0, keepdims=True)
    pos_h = np.transpose(pos, (2, 0, 1)).astype(np.float64)   # (H, i, j)

    # posA folds (1-mw)/4 so A = posA @ xp_sum = (1-mw)*mean_r;
    # posB is unscaled: B = posB @ (sB*xp_sum)^2 = (1-vw)*mean2_r.
    sA = ((1.0 - mw) / 4.0)
    posA = np.zeros((2, JT, HEADS, 2, JT), np.float32)
    posB = np.zeros((2, JT, HEADS, 2, JT), np.float32)
    for ic in range(2):
        for jc in range(2):
            blk = np.transpose(
                pos_h[:, ic * JT:(ic + 1) * JT, jc * JT:(jc + 1) * JT],
                (1, 0, 2))
            posB[ic, :, :, jc, :] = blk
            posA[ic, :, :, jc, :] = blk * sA[None, :, None]
    posA_bf = posA.astype(ml_dtypes.bfloat16)
    posB_bf = posB.astype(ml_dtypes.bfloat16)

    sB = (np.sqrt(1.0 - vw) / 4.0).astype(np.float32)
    sG = (np.sqrt(1.0 - vw) / (1.0 - mw)).astype(np.float32)
    return posA_bf, posB_bf, sB, sG, mw.astype(np.float32), vw.astype(np.float32)


def _raw_act(eng, out, in_, func, mybir, bias=0.0, scale=1.0):
    ins = [eng.lower_ap(in_)]
    ins.append(eng.lower_ap(bias) if not isinstance(bias, float)
               else mybir.ImmediateValue(dtype=mybir.dt.float32, value=bias))
    ins.append(mybir.ImmediateValue(dtype=mybir.dt.float32, value=scale))
    ins.append(mybir.ImmediateValue(dtype=mybir.dt.float32, value=0.0))
    return eng.add_instruction(
        mybir.InstActivation(
            name=eng.bass.get_next_instruction_name(),
            func=func, ins=ins, outs=[eng.lower_ap(out)]))


def _build_program(consts):
    _ensure_path()
    from contextlib import ExitStack
    import concourse.bass as bass  # noqa: F401
    import concourse.tile as tile
    from concourse import bacc, mybir

    posA_bf, posB_bf, sB, sG, mw, vw = consts
    eqh = bool(np.all(mw == mw[0]) and np.all(vw == vw[0]))
    assert eqh, "v5 kernel assumes per-head norm weights are equal"

    dt = mybir.dt.float32
    bt = mybir.dt.bfloat16
    AO = mybir.AluOpType
    AF = mybir.ActivationFunctionType
    AX = mybir.AxisListType

    nc = bacc.Bacc("TRN2", target_bir_lowering=False, debug=False,
                   enable_asserts=False)

    x_d = nc.dram_tensor("x", (BP, 14, 2, 14, 2, C), bt,
                         kind="ExternalInput").ap()
    pA_d = nc.dram_tensor("posA", (2, JT, HEADS, 2, JT), bt,
                          kind="ExternalInput").ap()
    pB_d = nc.dram_tensor("posB", (2, JT, HEADS, 2, JT), bt,
                          kind="ExternalInput").ap()
    out_d = nc.dram_tensor("out", (BP, 14, 2, 14, 2, C), bt,
                           kind="ExternalOutput").ap()

    x_re = x_d.transpose([0, 1, 3, 2, 4, 5])
    o_re = out_d.transpose([0, 1, 3, 2, 4, 5])

    with ExitStack() as ctx:
        tc = ctx.enter_context(tile.TileContext(nc))
        cpool = ctx.enter_context(tc.tile_pool(name="consts", bufs=1))
        xtp = ctx.enter_context(tc.tile_pool(name="xt", bufs=2))
        xnp = ctx.enter_context(tc.tile_pool(name="xn", bufs=2))
        xqp = ctx.enter_context(tc.tile_pool(name="xsq", bufs=2))
        smp = ctx.enter_context(tc.tile_pool(name="smalls", bufs=2))
        plp = ctx.enter_context(tc.tile_pool(name="pool", bufs=2))
        abp = ctx.enter_context(tc.tile_pool(name="ab", bufs=2))
        nmp = ctx.enter_context(tc.tile_pool(name="num", bufs=2))
        obp = ctx.enter_context(tc.tile_pool(name="outsb", bufs=2))
        ppp = ctx.enter_context(tc.tile_pool(name="ppsum", bufs=2,
                                             space="PSUM"))

        pA_sb, pB_sb = [], []
        for ic in range(2):
            tA = cpool.tile([JT, HEADS, 2, JT], bt, tag=f"posA{ic}")
            nc.sync.dma_start(tA[:], pA_d[ic])
            pA_sb.append(tA)
            tB = cpool.tile([JT, HEADS, 2, JT], bt, tag=f"posB{ic}")
            nc.sync.dma_start(tB[:], pB_d[ic])
            pB_sb.append(tB)

        for b in range(BP):
            xts, m2s = [], []
            # phase 1: load + xsq + halve + m2 (sum of squares), DVE 2x
            for jc in range(2):
                xt = xtp.tile([JT, 2, 2, C], bt, tag="xt")
                for d in range(2):
                    nc.sync.dma_start(xt[:, d],
                                      x_re[b, jc * 7:(jc + 1) * 7, :, d])
                xts.append(xt)
                xseg = xt[:].rearrange("p d s (h c) -> p (d s h) c", h=HEADS)
                xsq = xqp.tile([JT, 16, CH], bt, tag="xsq")
                nc.vector.tensor_mul(xsq[:], xseg, xseg)
                m2h = xqp.tile([JT, 16, CH // 2], bt, tag="m2h")
                nc.vector.tensor_add(m2h[:], xsq[:, :, 0:CH // 2],
                                     xsq[:, :, CH // 2:CH])
                m2 = smp.tile([JT, 16], dt, tag="m2")
                nc.vector.reduce_sum(m2[:], m2h[:], axis=AX.X)
                m2s.append(m2)

            # phase 2: S (ACT Rsqrt, grouped)
            Ss = []
            for jc in range(2):
                S = smp.tile([JT, 16], dt, tag="S")
                _raw_act(nc.scalar, S[:], m2s[jc][:], AF.Rsqrt, mybir,
                         bias=EPS, scale=1.0 / CH)
                Ss.append(S)

            # phase 3: xn on GpSimd; DVE: sxn, smalls, pooling
            xns, xpss, mmls, svls = [], [], [], []
            for jc in range(2):
                xt, m2, S = xts[jc], m2s[jc], Ss[jc]
                xseg = xt[:].rearrange("p d s (h c) -> p (d s h) c", h=HEADS)
                xn = xnp.tile([JT, 4, C], bt, tag="xn")
                nc.gpsimd.tensor_mul(
                    xn[:].rearrange("p q (h c) -> p (q h) c", h=HEADS),
                    xseg, S[:].unsqueeze(2).broadcast_to([JT, 16, CH]))
                xns.append(xn)

                xnh = xnp.tile([JT, 4, C // 2], bt, tag="xnh")
                nc.vector.tensor_add(xnh[:], xn[:, :, 0:C // 2],
                                     xn[:, :, C // 2:C])
                sxn = smp.tile([JT, 4], dt, tag="sxn")
                nc.vector.reduce_sum(sxn[:], xnh[:], axis=AX.X)

                r_ = smp.tile([JT, 16], dt, tag="r")
                nc.vector.tensor_mul(r_[:], S[:], S[:])
                u = smp.tile([JT, 16], dt, tag="u")
                nc.vector.tensor_mul(u[:], m2[:], r_[:])
                su = smp.tile([JT, 4], dt, tag="su")
                nc.vector.reduce_sum(
                    su[:], u[:].rearrange("p (q h) -> p q h", q=4), axis=AX.X)
                mml = smp.tile([JT, 4], dt, tag="mml")
                nc.vector.tensor_scalar_mul(mml[:], sxn[:], float(mw[0]) / C)
                T1v = smp.tile([JT, 4], dt, tag="T1v")
                nc.vector.scalar_tensor_tensor(
                    out=T1v[:], in0=mml[:],
                    scalar=float(-vw[0] * C / (C - 1.0) / (mw[0] * mw[0])),
                    in1=mml[:], op0=AO.mult, op1=AO.mult)
                svla = smp.tile([JT, 4], dt, tag="svla")
                nc.vector.tensor_scalar(
                    out=svla[:], in0=su[:],
                    scalar1=float(vw[0] / (C - 1.0)), scalar2=EPS,
                    op0=AO.mult, op1=AO.add)
                svl = smp.tile([JT, 4], dt, tag="svl")
                nc.vector.tensor_add(svl[:], svla[:], T1v[:])
                mmls.append(mml)
                svls.append(svl)

                tmp2 = plp.tile([JT, 2, C], bt, tag="tmp2")
                nc.vector.tensor_add(tmp2[:], xn[:, 0:2, :], xn[:, 2:4, :])
                xps = plp.tile([JT, C], bt, tag="xps")
                nc.vector.tensor_add(xps[:], tmp2[:, 0, :], tmp2[:, 1, :])
                xpss.append(xps)

            # phase 4: xpsq = (sB*xps)^2 (ACT Square, grouped)
            xpqs = []
            for jc in range(2):
                xpq = plp.tile([JT, C], bt, tag="xpq")
                nc.scalar.activation(xpq[:], xpss[jc][:], AF.Square,
                                     scale=float(sB[0]))
                xpqs.append(xpq)

            # phase 5: positional matmuls (A and B stationaries)
            pts = []
            for jc in range(2):
                pt = ppp.tile([JT, HEADS, 512], dt, tag="pt")
                for h in range(HEADS):
                    hs = slice(h * CH, (h + 1) * CH)
                    for ic in range(2):
                        nc.tensor.matmul(pt[:, h, 0:CH],
                                         pA_sb[ic][:, h, jc, :],
                                         xpss[ic][:, hs],
                                         start=(ic == 0), stop=(ic == 1))
                    for ic in range(2):
                        nc.tensor.matmul(pt[:, h, CH:2 * CH],
                                         pB_sb[ic][:, h, jc, :],
                                         xpqs[ic][:, hs],
                                         start=(ic == 0), stop=(ic == 1))
                pts.append(pt)

            # phase 6: sqa (ACT Square) + V0 (GpSimd) + Abq (ACT Identity)
            V0s, Abqs = [], []
            for jc in range(2):
                pt = pts[jc]
                sqa = abp.tile([JT, HEADS, CH], bt, tag="sqa")
                nc.scalar.activation(sqa[:], pt[:, :, 0:CH], AF.Square,
                                     scale=float(sG[0]))
                V0 = abp.tile([JT, HEADS, CH], bt, tag="V0")
                nc.vector.tensor_sub(V0[:], pt[:, :, CH:2 * CH], sqa[:])
                V0s.append(V0)
            for jc in range(2):
                pt = pts[jc]
                Abq = nmp.tile([JT, 4, C], bt, tag="Abq")
                for q in range(4):
                    nc.scalar.activation(
                        Abq[:, q, :].rearrange("p (h c) -> p h c", h=HEADS),
                        pt[:, :, 0:CH], AF.Identity,
                        bias=mmls[jc][:, q:q + 1])
                Abqs.append(Abq)

            # phase 7: iv (ACT Rsqrt, grouped)
            ivs = []
            for jc in range(2):
                V0f = V0s[jc][:].rearrange("p h c -> p (h c)")
                iv = nmp.tile([JT, 4, C], bt, tag="iv")
                for q in range(4):
                    _raw_act(nc.scalar, iv[:, q, :], V0f, AF.Rsqrt,
                             mybir, bias=svls[jc][:, q:q + 1])
                ivs.append(iv)

            # phase 8: num (DVE 2x) + out (DVE 2x) + DMA
            for jc in range(2):
                num = nmp.tile([JT, 4, C], bt, tag="num")
                nc.vector.tensor_sub(num[:], xns[jc][:], Abqs[jc][:])
                outsb = obp.tile([JT, 4, C], bt, tag="outsb")
                nc.vector.tensor_mul(
                    outsb[:].rearrange("p q c -> p (q c)"),
                    num[:].rearrange("p q c -> p (q c)"),
                    ivs[jc][:].rearrange("p q c -> p (q c)"))
                osb4 = outsb[:].rearrange("p (d s) c -> p d s c", d=2, s=2)
                for d in range(2):
                    nc.scalar.dma_start(o_re[b, jc * 7:(jc + 1) * 7, :, d],
                                        osb4[:, d])

    nc.compile()
    return nc


def _make_in_maps(inputs):
    import ml_dtypes
    x = np.asarray(inputs["x"], dtype=np.float32)
    cs = _host_consts(
        np.asarray(inputs["mean_norm_weight"], dtype=np.float32),
        np.asarray(inputs["var_norm_weight"], dtype=np.float32),
        np.asarray(inputs["pos_w"], dtype=np.float32),
        np.asarray(inputs["pos_b"], dtype=np.float32))
    posA_bf, posB_bf = cs[0], cs[1]
    x_bf = x.astype(ml_dtypes.bfloat16)
    in_maps = []
    for c in range(NCORES):
        m = {"posA": posA_bf, "posB": posB_bf,
             "x": np.ascontiguousarray(
                 x_bf[c * BP:(c + 1) * BP]).reshape(BP, 14, 2, 14, 2, C)}
        in_maps.append(m)
    return in_maps


def kernel(x, weight, bias, mean_norm_weight, var_norm_weight, pos_w, pos_b):
    _ensure_path()
    from concourse import bass_utils

    x = np.asarray(x, dtype=np.float32)
    B = x.shape[0]
    weight = np.asarray(weight, dtype=np.float32)
    bias = np.asarray(bias, dtype=np.float32)

    consts = _host_consts(
        np.asarray(mean_norm_weight, dtype=np.float32),
        np.asarray(var_norm_weight, dtype=np.float32),
        np.asarray(pos_w, dtype=np.float32),
        np.asarray(pos_b, dtype=np.float32))

    key = "v5"
    if key not in _PROGRAM_CACHE:
        _PROGRAM_CACHE[key] = _build_program(consts)
    nc = _PROGRAM_CACHE[key]

    in_maps = _make_in_maps(dict(
        x=x, mean_norm_weight=mean_norm_weight,
        var_norm_weight=var_norm_weight, pos_w=pos_w, pos_b=pos_b))

    res = bass_utils.run_bass_kernel_spmd(nc, in_maps,
                                          core_ids=list(range(NCORES)))
    out = np.concatenate(
        [np.asarray(res.results[c]["out"]).reshape(BP, T, C)
         for c in range(NCORES)], axis=0)
    assert out.shape == (B, T, C)
    out = out.astype(np.float32)

    if np.any(weight != 1.0):
        out = out * weight.reshape(1, 1, C)
    if np.any(bias != 0.0):
        out = out + bias.reshape(1, 1, C)
    return out


# revision 4
# speedup vs baseline: 1.4610x; 1.0266x over previous
"""Trainium2 Bass kernel for nn_DTN_47459388620856 (grouped-moment2 norm +
2x2 pooled positional-attention renormalization).

v5 — bf16 end-to-end + engine rebalance (HW-probed op rates):
  * x cast to bf16 on host; output returned bf16, upcast on host. Halves
    DMA traffic and makes every big DVE op eligible for the 2-byte 2x mode
    (probed: all-bf16 unit-stride tensor_tensor = 2x; broadcast/mixed = 1x;
    tensor_scalar with fp32 scalar-AP = 2x; reduce = 1x always).
  * DVE keeps only 2x-eligible big ops: xsq, halve-adds, reduces (halved
    first), pooling, num, out. ~23us/batch.
  * xn (S-broadcast mul, 1x anywhere) and V0 go to GpSimd.
  * ACT: S rsqrt, squares, Abq via Identity+bias-AP (no table load),
    iv via raw Rsqrt+bias-AP. Grouped: 2 table swaps/batch.
  * PE pos matmuls unchanged from v4.
"""

import numpy as np


def _ensure_path():
    try:
        import concourse  # noqa: F401
    except ImportError:
        import sys
        for p in ("/opt/trn_rl_repo",):
            if p not in sys.path:
                sys.path.insert(0, p)


EPS = 1e-5
HEADS, RES, PS = 4, 28, 14
T, C = RES * RES, 768
CH = C // HEADS
P = PS * PS
JT = 98
NCORES = 8
BP = 4

_PROGRAM_CACHE = {}


def _sigmoid(v):
    return 1.0 / (1.0 + np.exp(-v.astype(np.float64)))


def _host_consts(mean_norm_weight, var_norm_weight, pos_w, pos_b):
    import ml_dtypes
    mw = _sigmoid(mean_norm_weight)
    vw = _sigmoid(var_norm_weight)

    ind = np.arange(PS)[None, :] - np.arange(PS)[:, None]
    indx = np.tile(ind, (PS, PS))
    indy = np.repeat(np.repeat(ind, PS, axis=0), PS, axis=1)
    rel = np.stack([indx, indy, indx * indx + indy * indy], -1).astype(np.float32)
    scores = rel @ pos_w.T.astype(np.float32) + pos_b.astype(np.float32)
    e = np.exp(scores - scores.max(axis=0, keepdims=True))
    pos = e / e.sum(axis=0, keepdims=True)
    pos_h = np.transpose(pos, (2, 0, 1)).astype(np.float64)   # (H, i, j)

    # posA folds (1-mw)/4 so A = posA @ xp_sum = (1-mw)*mean_r;
    # posB is unscaled: B = posB @ (sB*xp_sum)^2 = (1-vw)*mean2_r.
    sA = ((1.0 - mw) / 4.0)
    posA = np.zeros((2, JT, HEADS, 2, JT), np.float32)
    posB = np.zeros((2, JT, HEADS, 2, JT), np.float32)
    for ic in range(2):
        for jc in range(2):
            blk = np.transpose(
                pos_h[:, ic * JT:(ic + 1) * JT, jc * JT:(jc + 1) * JT],
                (1, 0, 2))
            posB[ic, :, :, jc, :] = blk
            posA[ic, :, :, jc, :] = blk * sA[None, :, None]
    posA_bf = posA.astype(ml_dtypes.bfloat16)
    posB_bf = posB.astype(ml_dtypes.bfloat16)

    sB = (np.sqrt(1.0 - vw) / 4.0).astype(np.float32)
    sG = (np.sqrt(1.0 - vw) / (1.0 - mw)).astype(np.float32)
    return posA_bf, posB_bf, sB, sG, mw.astype(np.float32), vw.astype(np.float32)


def _raw_act(eng, out, in_, func, mybir, bias=0.0, scale=1.0):
    ins = [eng.lower_ap(in_)]
    ins.append(eng.lower_ap(bias) if not isinstance(bias, float)
               else mybir.ImmediateValue(dtype=mybir.dt.float32, value=bias))
    ins.append(mybir.ImmediateValue(dtype=mybir.dt.float32, value=scale))
    ins.append(mybir.ImmediateValue(dtype=mybir.dt.float32, value=0.0))
    return eng.add_instruction(
        mybir.InstActivation(
            name=eng.bass.get_next_instruction_name(),
            func=func, ins=ins, outs=[eng.lower_ap(out)]))


def _build_program(consts):
    _ensure_path()
    from contextlib import ExitStack
    import concourse.bass as bass  # noqa: F401
    import concourse.tile as tile
    from concourse import bacc, mybir

    posA_bf, posB_bf, sB, sG, mw, vw = consts
    eqh = bool(np.all(mw == mw[0]) and np.all(vw == vw[0]))
    assert eqh, "v5 kernel assumes per-head norm weights are equal"

    dt = mybir.dt.float32
    bt = mybir.dt.bfloat16
    AO = mybir.AluOpType
    AF = mybir.ActivationFunctionType
    AX = mybir.AxisListType

    nc = bacc.Bacc("TRN2", target_bir_lowering=False, debug=False,
                   enable_asserts=False)

    x_d = nc.dram_tensor("x", (BP, 14, 2, 14, 2, C), bt,
                         kind="ExternalInput").ap()
    pA_d = nc.dram_tensor("posA", (2, JT, HEADS, 2, JT), bt,
                          kind="ExternalInput").ap()
    pB_d = nc.dram_tensor("posB", (2, JT, HEADS, 2, JT), bt,
                          kind="ExternalInput").ap()
    out_d = nc.dram_tensor("out", (BP, 14, 2, 14, 2, C), bt,
                           kind="ExternalOutput").ap()

    x_re = x_d.transpose([0, 1, 3, 2, 4, 5])
    o_re = out_d.transpose([0, 1, 3, 2, 4, 5])

    with ExitStack() as ctx:
        tc = ctx.enter_context(tile.TileContext(nc))
        cpool = ctx.enter_context(tc.tile_pool(name="consts", bufs=1))
        xtp = ctx.enter_context(tc.tile_pool(name="xt", bufs=2))
        xnp = ctx.enter_context(tc.tile_pool(name="xn", bufs=2))
        xqp = ctx.enter_context(tc.tile_pool(name="xsq", bufs=2))
        smp = ctx.enter_context(tc.tile_pool(name="smalls", bufs=2))
        plp = ctx.enter_context(tc.tile_pool(name="pool", bufs=2))
        abp = ctx.enter_context(tc.tile_pool(name="ab", bufs=2))
        nmp = ctx.enter_context(tc.tile_pool(name="num", bufs=2))
        obp = ctx.enter_context(tc.tile_pool(name="outsb", bufs=2))
        ppp = ctx.enter_context(tc.tile_pool(name="ppsum", bufs=2,
                                             space="PSUM"))

        pA_sb, pB_sb = [], []
        for ic in range(2):
            tA = cpool.tile([JT, HEADS, 2, JT], bt, tag=f"posA{ic}")
            nc.sync.dma_start(tA[:], pA_d[ic])
            pA_sb.append(tA)
            tB = cpool.tile([JT, HEADS, 2, JT], bt, tag=f"posB{ic}")
            nc.sync.dma_start(tB[:], pB_d[ic])
            pB_sb.append(tB)

        for b in range(BP):
            xts, m2s = [], []
            # phase 1: load + xsq + halve + m2 (sum of squares), DVE 2x
            for jc in range(2):
                xt = xtp.tile([JT, 2, 2, C], bt, tag="xt")
                for d in range(2):
                    nc.sync.dma_start(xt[:, d],
                                      x_re[b, jc * 7:(jc + 1) * 7, :, d])
                xts.append(xt)
                xseg = xt[:].rearrange("p d s (h c) -> p (d s h) c", h=HEADS)
                xsq = xqp.tile([JT, 16, CH], bt, tag="xsq")
                nc.vector.tensor_mul(xsq[:], xseg, xseg)
                m2h = xqp.tile([JT, 16, CH // 2], bt, tag="m2h")
                nc.vector.tensor_add(m2h[:], xsq[:, :, 0:CH // 2],
                                     xsq[:, :, CH // 2:CH])
                m2 = smp.tile([JT, 16], dt, tag="m2")
                nc.vector.reduce_sum(m2[:], m2h[:], axis=AX.X)
                m2s.append(m2)

            # phase 2: S (ACT Rsqrt, grouped)
            Ss = []
            for jc in range(2):
                S = smp.tile([JT, 16], dt, tag="S")
                _raw_act(nc.scalar, S[:], m2s[jc][:], AF.Rsqrt, mybir,
                         bias=EPS, scale=1.0 / CH)
                Ss.append(S)

            # phase 3: xn on GpSimd; DVE: sxn, smalls, pooling
            xns, xpss, mmls, svls = [], [], [], []
            for jc in range(2):
                xt, m2, S = xts[jc], m2s[jc], Ss[jc]
                xn = xnp.tile([JT, 4, C], bt, tag="xn")
                for q in range(4):
                    d, s_ = divmod(q, 2)
                    for h in range(HEADS):
                        nc.vector.tensor_scalar_mul(
                            xn[:, q, h * CH:(h + 1) * CH],
                            xt[:, d, s_, h * CH:(h + 1) * CH],
                            S[:, q * HEADS + h:q * HEADS + h + 1])
                xns.append(xn)

                xnh = xnp.tile([JT, 4, C // 2], bt, tag="xnh")
                nc.vector.tensor_add(xnh[:], xn[:, :, 0:C // 2],
                                     xn[:, :, C // 2:C])
                sxn = smp.tile([JT, 4], dt, tag="sxn")
                nc.vector.reduce_sum(sxn[:], xnh[:], axis=AX.X)

                r_ = smp.tile([JT, 16], dt, tag="r")
                nc.vector.tensor_mul(r_[:], S[:], S[:])
                u = smp.tile([JT, 16], dt, tag="u")
                nc.vector.tensor_mul(u[:], m2[:], r_[:])
                su = smp.tile([JT, 4], dt, tag="su")
                nc.vector.reduce_sum(
                    su[:], u[:].rearrange("p (q h) -> p q h", q=4), axis=AX.X)
                mml = smp.tile([JT, 4], dt, tag="mml")
                nc.vector.tensor_scalar_mul(mml[:], sxn[:], float(mw[0]) / C)
                T1v = smp.tile([JT, 4], dt, tag="T1v")
                nc.vector.scalar_tensor_tensor(
                    out=T1v[:], in0=mml[:],
                    scalar=float(-vw[0] * C / (C - 1.0) / (mw[0] * mw[0])),
                    in1=mml[:], op0=AO.mult, op1=AO.mult)
                svla = smp.tile([JT, 4], dt, tag="svla")
                nc.vector.tensor_scalar(
                    out=svla[:], in0=su[:],
                    scalar1=float(vw[0] / (C - 1.0)), scalar2=EPS,
                    op0=AO.mult, op1=AO.add)
                svl = smp.tile([JT, 4], dt, tag="svl")
                nc.vector.tensor_add(svl[:], svla[:], T1v[:])
                mmls.append(mml)
                svls.append(svl)

                tmp2 = plp.tile([JT, 2, C], bt, tag="tmp2")
                nc.vector.tensor_add(tmp2[:], xn[:, 0:2, :], xn[:, 2:4, :])
                xps = plp.tile([JT, C], bt, tag="xps")
                nc.vector.tensor_add(xps[:], tmp2[:, 0, :], tmp2[:, 1, :])
                xpss.append(xps)

            # phase 4: xpsq = (sB*xps)^2 (ACT Square, grouped)
            xpqs = []
            for jc in range(2):
                xpq = plp.tile([JT, C], bt, tag="xpq")
                nc.scalar.activation(xpq[:], xpss[jc][:], AF.Square,
                                     scale=float(sB[0]))
                xpqs.append(xpq)

            # phase 5: positional matmuls (A and B stationaries)
            pts = []
            for jc in range(2):
                pt = ppp.tile([JT, HEADS, 512], dt, tag="pt")
                for h in range(HEADS):
                    hs = slice(h * CH, (h + 1) * CH)
                    for ic in range(2):
                        nc.tensor.matmul(pt[:, h, 0:CH],
                                         pA_sb[ic][:, h, jc, :],
                                         xpss[ic][:, hs],
                                         start=(ic == 0), stop=(ic == 1))
                    for ic in range(2):
                        nc.tensor.matmul(pt[:, h, CH:2 * CH],
                                         pB_sb[ic][:, h, jc, :],
                                         xpqs[ic][:, hs],
                                         start=(ic == 0), stop=(ic == 1))
                pts.append(pt)

            # phase 6: sqa (ACT Square) + V0 (GpSimd) + Abq (ACT Identity)
            V0s, Abqs = [], []
            for jc in range(2):
                pt = pts[jc]
                sqa = abp.tile([JT, HEADS, CH], bt, tag="sqa")
                nc.scalar.activation(sqa[:], pt[:, :, 0:CH], AF.Square,
                                     scale=float(sG[0]))
                V0 = abp.tile([JT, HEADS, CH], bt, tag="V0")
                nc.vector.tensor_sub(V0[:], pt[:, :, CH:2 * CH], sqa[:])
                V0s.append(V0)
            for jc in range(2):
                pt = pts[jc]
                Abq = nmp.tile([JT, 4, C], bt, tag="Abq")
                for q in range(4):
                    nc.scalar.activation(
                        Abq[:, q, :].rearrange("p (h c) -> p h c", h=HEADS),
                        pt[:, :, 0:CH], AF.Identity,
                        bias=mmls[jc][:, q:q + 1])
                Abqs.append(Abq)

            # phase 7: iv (ACT Rsqrt, grouped)
            ivs = []
            for jc in range(2):
                V0f = V0s[jc][:].rearrange("p h c -> p (h c)")
                iv = nmp.tile([JT, 4, C], bt, tag="iv")
                for q in range(4):
                    _raw_act(nc.scalar, iv[:, q, :], V0f, AF.Rsqrt,
                             mybir, bias=svls[jc][:, q:q + 1])
                ivs.append(iv)

            # phase 8: num (DVE 2x) + out (DVE 2x) + DMA
            for jc in range(2):
                num = nmp.tile([JT, 4, C], bt, tag="num")
                nc.vector.tensor_sub(num[:], xns[jc][:], Abqs[jc][:])
                outsb = obp.tile([JT, 4, C], bt, tag="outsb")
                nc.vector.tensor_mul(
                    outsb[:].rearrange("p q c -> p (q c)"),
                    num[:].rearrange("p q c -> p (q c)"),
                    ivs[jc][:].rearrange("p q c -> p (q c)"))
                osb4 = outsb[:].rearrange("p (d s) c -> p d s c", d=2, s=2)
                for d in range(2):
                    nc.scalar.dma_start(o_re[b, jc * 7:(jc + 1) * 7, :, d],
                                        osb4[:, d])

    nc.compile()
    return nc


def _make_in_maps(inputs):
    import ml_dtypes
    x = np.asarray(inputs["x"], dtype=np.float32)
    cs = _host_consts(
        np.asarray(inputs["mean_norm_weight"], dtype=np.float32),
        np.asarray(inputs["var_norm_weight"], dtype=np.float32),
        np.asarray(inputs["pos_w"], dtype=np.float32),
        np.asarray(inputs["pos_b"], dtype=np.float32))
    posA_bf, posB_bf = cs[0], cs[1]
    x_bf = x.astype(ml_dtypes.bfloat16)
    in_maps = []
    for c in range(NCORES):
        m = {"posA": posA_bf, "posB": posB_bf,
             "x": np.ascontiguousarray(
                 x_bf[c * BP:(c + 1) * BP]).reshape(BP, 14, 2, 14, 2, C)}
        in_maps.append(m)
    return in_maps


def kernel(x, weight, bias, mean_norm_weight, var_norm_weight, pos_w, pos_b):
    _ensure_path()
    from concourse import bass_utils

    x = np.asarray(x, dtype=np.float32)
    B = x.shape[0]
    weight = np.asarray(weight, dtype=np.float32)
    bias = np.asarray(bias, dtype=np.float32)

    consts = _host_consts(
        np.asarray(mean_norm_weight, dtype=np.float32),
        np.asarray(var_norm_weight, dtype=np.float32),
        np.asarray(pos_w, dtype=np.float32),
        np.asarray(pos_b, dtype=np.float32))

    key = "v5"
    if key not in _PROGRAM_CACHE:
        _PROGRAM_CACHE[key] = _build_program(consts)
    nc = _PROGRAM_CACHE[key]

    in_maps = _make_in_maps(dict(
        x=x, mean_norm_weight=mean_norm_weight,
        var_norm_weight=var_norm_weight, pos_w=pos_w, pos_b=pos_b))

    res = bass_utils.run_bass_kernel_spmd(nc, in_maps,
                                          core_ids=list(range(NCORES)))
    out = np.concatenate(
        [np.asarray(res.results[c]["out"]).reshape(BP, T, C)
         for c in range(NCORES)], axis=0)
    assert out.shape == (B, T, C)
    out = out.astype(np.float32)

    if np.any(weight != 1.0):
        out = out * weight.reshape(1, 1, C)
    if np.any(bias != 0.0):
        out = out + bias.reshape(1, 1, C)
    return out


# revision 5
# speedup vs baseline: 1.6864x; 1.1543x over previous
"""Trainium2 Bass kernel for nn_DTN_47459388620856 (grouped-moment2 norm +
2x2 pooled positional-attention renormalization).

v5 — bf16 end-to-end + engine rebalance (HW-probed op rates):
  * x cast to bf16 on host; output returned bf16, upcast on host. Halves
    DMA traffic and makes every big DVE op eligible for the 2-byte 2x mode
    (probed: all-bf16 unit-stride tensor_tensor = 2x; broadcast/mixed = 1x;
    tensor_scalar with fp32 scalar-AP = 2x; reduce = 1x always).
  * DVE keeps only 2x-eligible big ops: xsq, halve-adds, reduces (halved
    first), pooling, num, out. ~23us/batch.
  * xn (S-broadcast mul, 1x anywhere) and V0 go to GpSimd.
  * ACT: S rsqrt, squares, Abq via Identity+bias-AP (no table load),
    iv via raw Rsqrt+bias-AP. Grouped: 2 table swaps/batch.
  * PE pos matmuls unchanged from v4.
"""

import numpy as np


def _ensure_path():
    try:
        import concourse  # noqa: F401
    except ImportError:
        import sys
        for p in ("/opt/trn_rl_repo",):
            if p not in sys.path:
                sys.path.insert(0, p)


EPS = 1e-5
HEADS, RES, PS = 4, 28, 14
T, C = RES * RES, 768
CH = C // HEADS
P = PS * PS
JT = 98
NCORES = 8
BP = 4

_PROGRAM_CACHE = {}


def _sigmoid(v):
    return 1.0 / (1.0 + np.exp(-v.astype(np.float64)))


def _host_consts(mean_norm_weight, var_norm_weight, pos_w, pos_b):
    import ml_dtypes
    mw = _sigmoid(mean_norm_weight)
    vw = _sigmoid(var_norm_weight)

    ind = np.arange(PS)[None, :] - np.arange(PS)[:, None]
    indx = np.tile(ind, (PS, PS))
    indy = np.repeat(np.repeat(ind, PS, axis=0), PS, axis=1)
    rel = np.stack([indx, indy, indx * indx + indy * indy], -1).astype(np.float32)
    scores = rel @ pos_w.T.astype(np.float32) + pos_b.astype(np.float32)
    e = np.exp(scores - scores.max(axis=0, keepdims=True))
    pos = e / e.sum(axis=0, keepdims=True)
    pos_h = np.transpose(pos, (2, 0, 1)).astype(np.float64)   # (H, i, j)

    # posA folds (1-mw)/4 so A = posA @ xp_sum = (1-mw)*mean_r;
    # posB is unscaled: B = posB @ (sB*xp_sum)^2 = (1-vw)*mean2_r.
    sA = ((1.0 - mw) / 4.0)
    posA = np.zeros((2, JT, HEADS, 2, JT), np.float32)
    posB = np.zeros((2, JT, HEADS, 2, JT), np.float32)
    for ic in range(2):
        for jc in range(2):
            blk = np.transpose(
                pos_h[:, ic * JT:(ic + 1) * JT, jc * JT:(jc + 1) * JT],
                (1, 0, 2))
            posB[ic, :, :, jc, :] = blk
            posA[ic, :, :, jc, :] = blk * sA[None, :, None]
    posA_bf = posA.astype(ml_dtypes.bfloat16)
    posB_bf = posB.astype(ml_dtypes.bfloat16)

    sB = (np.sqrt(1.0 - vw) / 4.0).astype(np.float32)
    sG = (np.sqrt(1.0 - vw) / (1.0 - mw)).astype(np.float32)
    return posA_bf, posB_bf, sB, sG, mw.astype(np.float32), vw.astype(np.float32)


def _raw_act(eng, out, in_, func, mybir, bias=0.0, scale=1.0):
    ins = [eng.lower_ap(in_)]
    ins.append(eng.lower_ap(bias) if not isinstance(bias, float)
               else mybir.ImmediateValue(dtype=mybir.dt.float32, value=bias))
    ins.append(mybir.ImmediateValue(dtype=mybir.dt.float32, value=scale))
    ins.append(mybir.ImmediateValue(dtype=mybir.dt.float32, value=0.0))
    return eng.add_instruction(
        mybir.InstActivation(
            name=eng.bass.get_next_instruction_name(),
            func=func, ins=ins, outs=[eng.lower_ap(out)]))


def _build_program(consts):
    _ensure_path()
    from contextlib import ExitStack
    import concourse.bass as bass  # noqa: F401
    import concourse.tile as tile
    from concourse import bacc, mybir

    posA_bf, posB_bf, sB, sG, mw, vw = consts
    eqh = bool(np.all(mw == mw[0]) and np.all(vw == vw[0]))
    assert eqh, "v5 kernel assumes per-head norm weights are equal"

    dt = mybir.dt.float32
    bt = mybir.dt.bfloat16
    AO = mybir.AluOpType
    AF = mybir.ActivationFunctionType
    AX = mybir.AxisListType

    nc = bacc.Bacc("TRN2", target_bir_lowering=False, debug=False,
                   enable_asserts=False)

    x_d = nc.dram_tensor("x", (BP, 14, 2, 14, 2, C), bt,
                         kind="ExternalInput").ap()
    pA_d = nc.dram_tensor("posA", (2, JT, HEADS, 2, JT), bt,
                          kind="ExternalInput").ap()
    pB_d = nc.dram_tensor("posB", (2, JT, HEADS, 2, JT), bt,
                          kind="ExternalInput").ap()
    out_d = nc.dram_tensor("out", (BP, 14, 2, 14, 2, C), bt,
                           kind="ExternalOutput").ap()

    x_re = x_d.transpose([0, 1, 3, 2, 4, 5])
    o_re = out_d.transpose([0, 1, 3, 2, 4, 5])

    with ExitStack() as ctx:
        tc = ctx.enter_context(tile.TileContext(nc))
        cpool = ctx.enter_context(tc.tile_pool(name="consts", bufs=1))
        xtp = ctx.enter_context(tc.tile_pool(name="xt", bufs=2))
        xnp = ctx.enter_context(tc.tile_pool(name="xn", bufs=2))
        xqp = ctx.enter_context(tc.tile_pool(name="xsq", bufs=2))
        smp = ctx.enter_context(tc.tile_pool(name="smalls", bufs=2))
        plp = ctx.enter_context(tc.tile_pool(name="pool", bufs=2))
        abp = ctx.enter_context(tc.tile_pool(name="ab", bufs=2))
        nmp = ctx.enter_context(tc.tile_pool(name="num", bufs=2))
        obp = ctx.enter_context(tc.tile_pool(name="outsb", bufs=2))
        ppp = ctx.enter_context(tc.tile_pool(name="ppsum", bufs=2,
                                             space="PSUM"))

        pA_sb, pB_sb = [], []
        for ic in range(2):
            tA = cpool.tile([JT, HEADS, 2, JT], bt, tag=f"posA{ic}")
            nc.sync.dma_start(tA[:], pA_d[ic])
            pA_sb.append(tA)
            tB = cpool.tile([JT, HEADS, 2, JT], bt, tag=f"posB{ic}")
            nc.sync.dma_start(tB[:], pB_d[ic])
            pB_sb.append(tB)

        # Software-pipelined: iteration `it` emits batch `it`'s front half
        # (loads, xsq/m2, S, xn/pool, matmuls) interleaved with batch
        # `it-1`'s back half (sqa/V0, Abq, iv, num/out, stores), so neither
        # the in-order DVE nor the in-order ACT stream ever stalls long.
        prev = None
        for it in range(BP + 1):
            b = it
            # -- prev back-half part 1: sqa (ACT, reads PSUM A-region) + V0
            if prev is not None:
                for jc in range(2):
                    pt = prev["pts"][jc]
                    sqa = abp.tile([JT, HEADS, CH], bt, tag="sqa")
                    nc.scalar.activation(sqa[:], pt[:, :, 0:CH], AF.Square,
                                         scale=float(sG[0]))
                    V0 = abp.tile([JT, HEADS, CH], bt, tag="V0")
                    nc.vector.tensor_sub(V0[:], pt[:, :, CH:2 * CH], sqa[:])
                    prev[f"V0{jc}"] = V0

            cur = None
            if b < BP:
                cur = {"xts": [], "m2s": [], "Ss": [], "xns": [],
                       "xpss": [], "xpqs": [], "mmls": [], "svls": [],
                       "pts": [], "b": b}
                # front 1: loads + xsq + halve + m2 (DVE 2x)
                for jc in range(2):
                    xt = xtp.tile([JT, 2, 2, C], bt, tag="xt")
                    for d in range(2):
                        nc.sync.dma_start(xt[:, d],
                                          x_re[b, jc * 7:(jc + 1) * 7, :, d])
                    cur["xts"].append(xt)
                    xseg = xt[:].rearrange("p d s (h c) -> p (d s h) c",
                                           h=HEADS)
                    xsq = xqp.tile([JT, 16, CH], bt, tag="xsq")
                    nc.vector.tensor_mul(xsq[:], xseg, xseg)
                    m2h = xqp.tile([JT, 16, CH // 2], bt, tag="m2h")
                    nc.vector.tensor_add(m2h[:], xsq[:, :, 0:CH // 2],
                                         xsq[:, :, CH // 2:CH])
                    m2 = smp.tile([JT, 16], dt, tag="m2")
                    nc.vector.reduce_sum(m2[:], m2h[:], axis=AX.X)
                    cur["m2s"].append(m2)
                # front 2: S (ACT Rsqrt)
                for jc in range(2):
                    S = smp.tile([JT, 16], dt, tag="S")
                    _raw_act(nc.scalar, S[:], cur["m2s"][jc][:], AF.Rsqrt,
                             mybir, bias=EPS, scale=1.0 / CH)
                    cur["Ss"].append(S)

            # -- prev back-half part 2: Abq (ACT Identity x8)
            if prev is not None:
                for jc in range(2):
                    pt = prev["pts"][jc]
                    Abq = nmp.tile([JT, 4, C], bt, tag="Abq")
                    for q in range(4):
                        nc.scalar.activation(
                            Abq[:, q, :].rearrange("p (h c) -> p h c",
                                                   h=HEADS),
                            pt[:, :, 0:CH], AF.Identity,
                            bias=prev["mmls"][jc][:, q:q + 1])
                    prev[f"Abq{jc}"] = Abq

            if cur is not None:
                # front 3: xn (DVE ts_mul 2x) + sxn + smalls + pooling
                for jc in range(2):
                    xt, m2, S = (cur["xts"][jc], cur["m2s"][jc],
                                 cur["Ss"][jc])
                    xn = xnp.tile([JT, 4, C], bt, tag="xn")
                    for q in range(4):
                        d, s_ = divmod(q, 2)
                        for h in range(HEADS):
                            nc.vector.tensor_scalar_mul(
                                xn[:, q, h * CH:(h + 1) * CH],
                                xt[:, d, s_, h * CH:(h + 1) * CH],
                                S[:, q * HEADS + h:q * HEADS + h + 1])
                    cur["xns"].append(xn)

                    xnh = xnp.tile([JT, 4, C // 2], bt, tag="xnh")
                    nc.vector.tensor_add(xnh[:], xn[:, :, 0:C // 2],
                                         xn[:, :, C // 2:C])
                    sxn = smp.tile([JT, 4], dt, tag="sxn")
                    nc.vector.reduce_sum(sxn[:], xnh[:], axis=AX.X)

                    r_ = smp.tile([JT, 16], dt, tag="r")
                    nc.vector.tensor_mul(r_[:], S[:], S[:])
                    u = smp.tile([JT, 16], dt, tag="u")
                    nc.vector.tensor_mul(u[:], m2[:], r_[:])
                    su = smp.tile([JT, 4], dt, tag="su")
                    nc.vector.reduce_sum(
                        su[:], u[:].rearrange("p (q h) -> p q h", q=4),
                        axis=AX.X)
                    mml = smp.tile([JT, 4], dt, tag="mml")
                    nc.vector.tensor_scalar_mul(mml[:], sxn[:],
                                                float(mw[0]) / C)
                    T1v = smp.tile([JT, 4], dt, tag="T1v")
                    nc.vector.scalar_tensor_tensor(
                        out=T1v[:], in0=mml[:],
                        scalar=float(-vw[0] * C / (C - 1.0)
                                     / (mw[0] * mw[0])),
                        in1=mml[:], op0=AO.mult, op1=AO.mult)
                    svla = smp.tile([JT, 4], dt, tag="svla")
                    nc.vector.tensor_scalar(
                        out=svla[:], in0=su[:],
                        scalar1=float(vw[0] / (C - 1.0)), scalar2=EPS,
                        op0=AO.mult, op1=AO.add)
                    svl = smp.tile([JT, 4], dt, tag="svl")
                    nc.vector.tensor_add(svl[:], svla[:], T1v[:])
                    cur["mmls"].append(mml)
                    cur["svls"].append(svl)

                    tmp2 = plp.tile([JT, 2, C], bt, tag="tmp2")
                    nc.vector.tensor_add(tmp2[:], xn[:, 0:2, :],
                                         xn[:, 2:4, :])
                    xps = plp.tile([JT, C], bt, tag="xps")
                    nc.vector.tensor_add(xps[:], tmp2[:, 0, :],
                                         tmp2[:, 1, :])
                    cur["xpss"].append(xps)

            # -- prev back-half part 3: iv (ACT Rsqrt x8)
            if prev is not None:
                for jc in range(2):
                    V0f = prev[f"V0{jc}"][:].rearrange("p h c -> p (h c)")
                    iv = nmp.tile([JT, 4, C], bt, tag="iv")
                    for q in range(4):
                        _raw_act(nc.scalar, iv[:, q, :], V0f, AF.Rsqrt,
                                 mybir, bias=prev["svls"][jc][:, q:q + 1])
                    prev[f"iv{jc}"] = iv

            if cur is not None:
                # front 4: A-side matmuls (need only xps), then xpq (ACT
                # Square), then B-side matmuls.
                for jc in range(2):
                    pt = ppp.tile([JT, HEADS, 512], dt, tag="pt")
                    cur["pts"].append(pt)
                for jc in range(2):
                    pt = cur["pts"][jc]
                    for h in range(HEADS):
                        hs = slice(h * CH, (h + 1) * CH)
                        for ic in range(2):
                            nc.tensor.matmul(pt[:, h, 0:CH],
                                             pA_sb[ic][:, h, jc, :],
                                             cur["xpss"][ic][:, hs],
                                             start=(ic == 0), stop=(ic == 1))
                for jc in range(2):
                    xpq = plp.tile([JT, C], bt, tag="xpq")
                    nc.scalar.activation(xpq[:], cur["xpss"][jc][:],
                                         AF.Square, scale=float(sB[0]))
                    cur["xpqs"].append(xpq)
                for jc in range(2):
                    pt = cur["pts"][jc]
                    for h in range(HEADS):
                        hs = slice(h * CH, (h + 1) * CH)
                        for ic in range(2):
                            nc.tensor.matmul(pt[:, h, CH:2 * CH],
                                             pB_sb[ic][:, h, jc, :],
                                             cur["xpqs"][ic][:, hs],
                                             start=(ic == 0), stop=(ic == 1))

            # -- prev back-half part 4: num + out (DVE 2x) + stores
            if prev is not None:
                pb = prev["b"]
                for jc in range(2):
                    num = nmp.tile([JT, 4, C], bt, tag="num")
                    nc.vector.tensor_sub(num[:], prev["xns"][jc][:],
                                         prev[f"Abq{jc}"][:])
                    outsb = obp.tile([JT, 4, C], bt, tag="outsb")
                    nc.vector.tensor_mul(
                        outsb[:].rearrange("p q c -> p (q c)"),
                        num[:].rearrange("p q c -> p (q c)"),
                        prev[f"iv{jc}"][:].rearrange("p q c -> p (q c)"))
                    osb4 = outsb[:].rearrange("p (d s) c -> p d s c",
                                              d=2, s=2)
                    for d in range(2):
                        nc.gpsimd.dma_start(
                            o_re[pb, jc * 7:(jc + 1) * 7, :, d], osb4[:, d])

            prev = cur

    nc.compile()
    return nc


def _make_in_maps(inputs):
    import ml_dtypes
    x = np.asarray(inputs["x"], dtype=np.float32)
    cs = _host_consts(
        np.asarray(inputs["mean_norm_weight"], dtype=np.float32),
        np.asarray(inputs["var_norm_weight"], dtype=np.float32),
        np.asarray(inputs["pos_w"], dtype=np.float32),
        np.asarray(inputs["pos_b"], dtype=np.float32))
    posA_bf, posB_bf = cs[0], cs[1]
    x_bf = x.astype(ml_dtypes.bfloat16)
    in_maps = []
    for c in range(NCORES):
        m = {"posA": posA_bf, "posB": posB_bf,
             "x": np.ascontiguousarray(
                 x_bf[c * BP:(c + 1) * BP]).reshape(BP, 14, 2, 14, 2, C)}
        in_maps.append(m)
    return in_maps


def kernel(x, weight, bias, mean_norm_weight, var_norm_weight, pos_w, pos_b):
    _ensure_path()
    from concourse import bass_utils

    x = np.asarray(x, dtype=np.float32)
    B = x.shape[0]
    weight = np.asarray(weight, dtype=np.float32)
    bias = np.asarray(bias, dtype=np.float32)

    consts = _host_consts(
        np.asarray(mean_norm_weight, dtype=np.float32),
        np.asarray(var_norm_weight, dtype=np.float32),
        np.asarray(pos_w, dtype=np.float32),
        np.asarray(pos_b, dtype=np.float32))

    key = "v5"
    if key not in _PROGRAM_CACHE:
        _PROGRAM_CACHE[key] = _build_program(consts)
    nc = _PROGRAM_CACHE[key]

    in_maps = _make_in_maps(dict(
        x=x, mean_norm_weight=mean_norm_weight,
        var_norm_weight=var_norm_weight, pos_w=pos_w, pos_b=pos_b))

    res = bass_utils.run_bass_kernel_spmd(nc, in_maps,
                                          core_ids=list(range(NCORES)))
    out = np.concatenate(
        [np.asarray(res.results[c]["out"]).reshape(BP, T, C)
         for c in range(NCORES)], axis=0)
    assert out.shape == (B, T, C)
    out = out.astype(np.float32)

    if np.any(weight != 1.0):
        out = out * weight.reshape(1, 1, C)
    if np.any(bias != 0.0):
        out = out + bias.reshape(1, 1, C)
    return out


# revision 6
# speedup vs baseline: 1.7103x; 1.0142x over previous
"""Trainium2 Bass kernel for nn_DTN_47459388620856 (grouped-moment2 norm +
2x2 pooled positional-attention renormalization).

v5 — bf16 end-to-end + engine rebalance (HW-probed op rates):
  * x cast to bf16 on host; output returned bf16, upcast on host. Halves
    DMA traffic and makes every big DVE op eligible for the 2-byte 2x mode
    (probed: all-bf16 unit-stride tensor_tensor = 2x; broadcast/mixed = 1x;
    tensor_scalar with fp32 scalar-AP = 2x; reduce = 1x always).
  * DVE keeps only 2x-eligible big ops: xsq, halve-adds, reduces (halved
    first), pooling, num, out. ~23us/batch.
  * xn (S-broadcast mul, 1x anywhere) and V0 go to GpSimd.
  * ACT: S rsqrt, squares, Abq via Identity+bias-AP (no table load),
    iv via raw Rsqrt+bias-AP. Grouped: 2 table swaps/batch.
  * PE pos matmuls unchanged from v4.
"""

import numpy as np


def _ensure_path():
    try:
        import concourse  # noqa: F401
    except ImportError:
        import sys
        for p in ("/opt/trn_rl_repo",):
            if p not in sys.path:
                sys.path.insert(0, p)


EPS = 1e-5
HEADS, RES, PS = 4, 28, 14
T, C = RES * RES, 768
CH = C // HEADS
P = PS * PS
JT = 98
NCORES = 8
BP = 4

_PROGRAM_CACHE = {}


def _sigmoid(v):
    return 1.0 / (1.0 + np.exp(-v.astype(np.float64)))


def _host_consts(mean_norm_weight, var_norm_weight, pos_w, pos_b):
    import ml_dtypes
    mw = _sigmoid(mean_norm_weight)
    vw = _sigmoid(var_norm_weight)

    ind = np.arange(PS)[None, :] - np.arange(PS)[:, None]
    indx = np.tile(ind, (PS, PS))
    indy = np.repeat(np.repeat(ind, PS, axis=0), PS, axis=1)
    rel = np.stack([indx, indy, indx * indx + indy * indy], -1).astype(np.float32)
    scores = rel @ pos_w.T.astype(np.float32) + pos_b.astype(np.float32)
    e = np.exp(scores - scores.max(axis=0, keepdims=True))
    pos = e / e.sum(axis=0, keepdims=True)
    pos_h = np.transpose(pos, (2, 0, 1)).astype(np.float64)   # (H, i, j)

    # posA folds (1-mw)/4 so A = posA @ xp_sum = (1-mw)*mean_r;
    # posB is unscaled: B = posB @ (sB*xp_sum)^2 = (1-vw)*mean2_r.
    sA = ((1.0 - mw) / 4.0)
    posA = np.zeros((2, JT, HEADS, 2, JT), np.float32)
    posB = np.zeros((2, JT, HEADS, 2, JT), np.float32)
    for ic in range(2):
        for jc in range(2):
            blk = np.transpose(
                pos_h[:, ic * JT:(ic + 1) * JT, jc * JT:(jc + 1) * JT],
                (1, 0, 2))
            posB[ic, :, :, jc, :] = blk
            posA[ic, :, :, jc, :] = blk * sA[None, :, None]
    posA_bf = posA.astype(ml_dtypes.bfloat16)
    posB_bf = posB.astype(ml_dtypes.bfloat16)

    sB = (np.sqrt(1.0 - vw) / 4.0).astype(np.float32)
    sG = (np.sqrt(1.0 - vw) / (1.0 - mw)).astype(np.float32)
    return posA_bf, posB_bf, sB, sG, mw.astype(np.float32), vw.astype(np.float32)


def _raw_act(eng, out, in_, func, mybir, bias=0.0, scale=1.0):
    ins = [eng.lower_ap(in_)]
    ins.append(eng.lower_ap(bias) if not isinstance(bias, float)
               else mybir.ImmediateValue(dtype=mybir.dt.float32, value=bias))
    ins.append(mybir.ImmediateValue(dtype=mybir.dt.float32, value=scale))
    ins.append(mybir.ImmediateValue(dtype=mybir.dt.float32, value=0.0))
    return eng.add_instruction(
        mybir.InstActivation(
            name=eng.bass.get_next_instruction_name(),
            func=func, ins=ins, outs=[eng.lower_ap(out)]))


def _build_program(consts):
    _ensure_path()
    from contextlib import ExitStack
    import concourse.bass as bass  # noqa: F401
    import concourse.tile as tile
    from concourse import bacc, mybir

    posA_bf, posB_bf, sB, sG, mw, vw = consts
    eqh = bool(np.all(mw == mw[0]) and np.all(vw == vw[0]))
    assert eqh, "v5 kernel assumes per-head norm weights are equal"

    dt = mybir.dt.float32
    bt = mybir.dt.bfloat16
    AO = mybir.AluOpType
    AF = mybir.ActivationFunctionType
    AX = mybir.AxisListType

    nc = bacc.Bacc("TRN2", target_bir_lowering=False, debug=False,
                   enable_asserts=False)

    x_d = nc.dram_tensor("x", (BP, 14, 2, 14, 2, C), bt,
                         kind="ExternalInput").ap()
    pA_d = nc.dram_tensor("posA", (2, JT, HEADS, 2, JT), bt,
                          kind="ExternalInput").ap()
    pB_d = nc.dram_tensor("posB", (2, JT, HEADS, 2, JT), bt,
                          kind="ExternalInput").ap()
    out_d = nc.dram_tensor("out", (BP, 14, 2, 14, 2, C), bt,
                           kind="ExternalOutput").ap()

    x_re = x_d.transpose([0, 1, 3, 2, 4, 5])
    o_re = out_d.transpose([0, 1, 3, 2, 4, 5])

    with ExitStack() as ctx:
        tc = ctx.enter_context(tile.TileContext(nc))
        cpool = ctx.enter_context(tc.tile_pool(name="consts", bufs=1))
        xtp = ctx.enter_context(tc.tile_pool(name="xt", bufs=2))
        xnp = ctx.enter_context(tc.tile_pool(name="xn", bufs=2))
        xqp = ctx.enter_context(tc.tile_pool(name="xsq", bufs=2))
        smp = ctx.enter_context(tc.tile_pool(name="smalls", bufs=2))
        plp = ctx.enter_context(tc.tile_pool(name="pool", bufs=2))
        abp = ctx.enter_context(tc.tile_pool(name="ab", bufs=2))
        nmp = ctx.enter_context(tc.tile_pool(name="num", bufs=2))
        obp = ctx.enter_context(tc.tile_pool(name="outsb", bufs=2))
        ppp = ctx.enter_context(tc.tile_pool(name="ppsum", bufs=2,
                                             space="PSUM"))

        pA_sb, pB_sb = [], []
        for ic in range(2):
            tA = cpool.tile([JT, HEADS, 2, JT], bt, tag=f"posA{ic}")
            nc.sync.dma_start(tA[:], pA_d[ic])
            pA_sb.append(tA)
            tB = cpool.tile([JT, HEADS, 2, JT], bt, tag=f"posB{ic}")
            nc.sync.dma_start(tB[:], pB_d[ic])
            pB_sb.append(tB)

        # Software-pipelined: iteration `it` emits batch `it`'s front half
        # (loads, xsq/m2, S, xn/pool, matmuls) interleaved with batch
        # `it-1`'s back half (sqa/V0, Abq, iv, num/out, stores), so neither
        # the in-order DVE nor the in-order ACT stream ever stalls long.
        prev = None
        for it in range(BP + 1):
            b = it
            # -- prev back-half part 1: sqa (ACT, reads PSUM A-region) + V0
            if prev is not None:
                for jc in range(2):
                    pt = prev["pts"][jc]
                    sqa = abp.tile([JT, HEADS, CH], bt, tag="sqa")
                    nc.scalar.activation(sqa[:], pt[:, :, 0:CH], AF.Square,
                                         scale=float(sG[0]))
                    V0 = abp.tile([JT, HEADS, CH], bt, tag="V0")
                    nc.vector.tensor_sub(V0[:], pt[:, :, CH:2 * CH], sqa[:])
                    prev[f"V0{jc}"] = V0

            cur = None
            if b < BP:
                cur = {"xts": [], "m2s": [], "Ss": [], "xns": [],
                       "xpss": [], "xpqs": [], "mmls": [], "svls": [],
                       "pts": [], "b": b}
                # front 1: loads + xsq + halve + m2 (DVE 2x)
                for jc in range(2):
                    xt = xtp.tile([JT, 2, 2, C], bt, tag="xt")
                    for d in range(2):
                        nc.sync.dma_start(xt[:, d],
                                          x_re[b, jc * 7:(jc + 1) * 7, :, d])
                    cur["xts"].append(xt)
                    xseg = xt[:].rearrange("p d s (h c) -> p (d s h) c",
                                           h=HEADS)
                    xsq = xqp.tile([JT, 16, CH], bt, tag="xsq")
                    nc.scalar.activation(xsq[:], xseg, AF.Square)
                    m2h = xqp.tile([JT, 16, CH // 2], bt, tag="m2h")
                    nc.vector.tensor_add(m2h[:], xsq[:, :, 0:CH // 2],
                                         xsq[:, :, CH // 2:CH])
                    m2q = xqp.tile([JT, 16, CH // 4], bt, tag="m2q")
                    nc.vector.tensor_add(m2q[:], m2h[:, :, 0:CH // 4],
                                         m2h[:, :, CH // 4:CH // 2])
                    m2 = smp.tile([JT, 16], dt, tag="m2")
                    nc.vector.reduce_sum(m2[:], m2q[:], axis=AX.X)
                    cur["m2s"].append(m2)
                # front 2: S (ACT Rsqrt)
                for jc in range(2):
                    S = smp.tile([JT, 16], dt, tag="S")
                    _raw_act(nc.scalar, S[:], cur["m2s"][jc][:], AF.Rsqrt,
                             mybir, bias=EPS, scale=1.0 / CH)
                    cur["Ss"].append(S)

            # -- prev back-half part 2: Abq (ACT Identity x8)
            if prev is not None:
                for jc in range(2):
                    pt = prev["pts"][jc]
                    Abq = nmp.tile([JT, 4, C], bt, tag="Abq")
                    for q in range(4):
                        nc.scalar.activation(
                            Abq[:, q, :].rearrange("p (h c) -> p h c",
                                                   h=HEADS),
                            pt[:, :, 0:CH], AF.Identity,
                            bias=prev["mmls"][jc][:, q:q + 1])
                    prev[f"Abq{jc}"] = Abq

            if cur is not None:
                # front 3: xn (DVE ts_mul 2x) + sxn + smalls + pooling
                for jc in range(2):
                    xt, m2, S = (cur["xts"][jc], cur["m2s"][jc],
                                 cur["Ss"][jc])
                    xn = xnp.tile([JT, 4, C], bt, tag="xn")
                    for q in range(4):
                        d, s_ = divmod(q, 2)
                        for h in range(HEADS):
                            nc.vector.tensor_scalar_mul(
                                xn[:, q, h * CH:(h + 1) * CH],
                                xt[:, d, s_, h * CH:(h + 1) * CH],
                                S[:, q * HEADS + h:q * HEADS + h + 1])
                    cur["xns"].append(xn)

                    xnh = xnp.tile([JT, 4, C // 2], bt, tag="xnh")
                    nc.vector.tensor_add(xnh[:], xn[:, :, 0:C // 2],
                                         xn[:, :, C // 2:C])
                    xnq = xnp.tile([JT, 4, C // 4], bt, tag="xnq")
                    nc.vector.tensor_add(xnq[:], xnh[:, :, 0:C // 4],
                                         xnh[:, :, C // 4:C // 2])
                    sxn = smp.tile([JT, 4], dt, tag="sxn")
                    nc.vector.reduce_sum(sxn[:], xnq[:], axis=AX.X)

                    r_ = smp.tile([JT, 16], dt, tag="r")
                    nc.vector.tensor_mul(r_[:], S[:], S[:])
                    u = smp.tile([JT, 16], dt, tag="u")
                    nc.vector.tensor_mul(u[:], m2[:], r_[:])
                    su = smp.tile([JT, 4], dt, tag="su")
                    nc.vector.reduce_sum(
                        su[:], u[:].rearrange("p (q h) -> p q h", q=4),
                        axis=AX.X)
                    mml = smp.tile([JT, 4], dt, tag="mml")
                    nc.vector.tensor_scalar_mul(mml[:], sxn[:],
                                                float(mw[0]) / C)
                    T1v = smp.tile([JT, 4], dt, tag="T1v")
                    nc.vector.scalar_tensor_tensor(
                        out=T1v[:], in0=mml[:],
                        scalar=float(-vw[0] * C / (C - 1.0)
                                     / (mw[0] * mw[0])),
                        in1=mml[:], op0=AO.mult, op1=AO.mult)
                    svla = smp.tile([JT, 4], dt, tag="svla")
                    nc.vector.tensor_scalar(
                        out=svla[:], in0=su[:],
                        scalar1=float(vw[0] / (C - 1.0)), scalar2=EPS,
                        op0=AO.mult, op1=AO.add)
                    svl = smp.tile([JT, 4], dt, tag="svl")
                    nc.vector.tensor_add(svl[:], svla[:], T1v[:])
                    cur["mmls"].append(mml)
                    cur["svls"].append(svl)

                    tmp2 = plp.tile([JT, 2, C], bt, tag="tmp2")
                    nc.vector.tensor_add(tmp2[:], xn[:, 0:2, :],
                                         xn[:, 2:4, :])
                    xps = plp.tile([JT, C], bt, tag="xps")
                    nc.vector.tensor_add(xps[:], tmp2[:, 0, :],
                                         tmp2[:, 1, :])
                    cur["xpss"].append(xps)

            # -- prev back-half part 3: iv (ACT Rsqrt x8)
            if prev is not None:
                for jc in range(2):
                    V0f = prev[f"V0{jc}"][:].rearrange("p h c -> p (h c)")
                    iv = nmp.tile([JT, 4, C], bt, tag="iv")
                    for q in range(4):
                        _raw_act(nc.scalar, iv[:, q, :], V0f, AF.Rsqrt,
                                 mybir, bias=prev["svls"][jc][:, q:q + 1])
                    prev[f"iv{jc}"] = iv

            if cur is not None:
                # front 4: A-side matmuls (need only xps), then xpq (ACT
                # Square), then B-side matmuls.
                for jc in range(2):
                    pt = ppp.tile([JT, HEADS, 512], dt, tag="pt")
                    cur["pts"].append(pt)
                for jc in range(2):
                    pt = cur["pts"][jc]
                    for h in range(HEADS):
                        hs = slice(h * CH, (h + 1) * CH)
                        for ic in range(2):
                            nc.tensor.matmul(pt[:, h, 0:CH],
                                             pA_sb[ic][:, h, jc, :],
                                             cur["xpss"][ic][:, hs],
                                             start=(ic == 0), stop=(ic == 1))
                for jc in range(2):
                    xpq = plp.tile([JT, C], bt, tag="xpq")
                    nc.scalar.activation(xpq[:], cur["xpss"][jc][:],
                                         AF.Square, scale=float(sB[0]))
                    cur["xpqs"].append(xpq)
                for jc in range(2):
                    pt = cur["pts"][jc]
                    for h in range(HEADS):
                        hs = slice(h * CH, (h + 1) * CH)
                        for ic in range(2):
                            nc.tensor.matmul(pt[:, h, CH:2 * CH],
                                             pB_sb[ic][:, h, jc, :],
                                             cur["xpqs"][ic][:, hs],
                                             start=(ic == 0), stop=(ic == 1))

            # -- prev back-half part 4: num + out (DVE 2x) + stores
            if prev is not None:
                pb = prev["b"]
                for jc in range(2):
                    num = nmp.tile([JT, 4, C], bt, tag="num")
                    nc.vector.tensor_sub(num[:], prev["xns"][jc][:],
                                         prev[f"Abq{jc}"][:])
                    outsb = obp.tile([JT, 4, C], bt, tag="outsb")
                    nc.vector.tensor_mul(
                        outsb[:].rearrange("p q c -> p (q c)"),
                        num[:].rearrange("p q c -> p (q c)"),
                        prev[f"iv{jc}"][:].rearrange("p q c -> p (q c)"))
                    osb4 = outsb[:].rearrange("p (d s) c -> p d s c",
                                              d=2, s=2)
                    for d in range(2):
                        nc.gpsimd.dma_start(
                            o_re[pb, jc * 7:(jc + 1) * 7, :, d], osb4[:, d])

            prev = cur

    nc.compile()
    return nc


def _make_in_maps(inputs):
    import ml_dtypes
    x = np.asarray(inputs["x"], dtype=np.float32)
    cs = _host_consts(
        np.asarray(inputs["mean_norm_weight"], dtype=np.float32),
        np.asarray(inputs["var_norm_weight"], dtype=np.float32),
        np.asarray(inputs["pos_w"], dtype=np.float32),
        np.asarray(inputs["pos_b"], dtype=np.float32))
    posA_bf, posB_bf = cs[0], cs[1]
    x_bf = x.astype(ml_dtypes.bfloat16)
    in_maps = []
    for c in range(NCORES):
        m = {"posA": posA_bf, "posB": posB_bf,
             "x": np.ascontiguousarray(
                 x_bf[c * BP:(c + 1) * BP]).reshape(BP, 14, 2, 14, 2, C)}
        in_maps.append(m)
    return in_maps


def kernel(x, weight, bias, mean_norm_weight, var_norm_weight, pos_w, pos_b):
    _ensure_path()
    from concourse import bass_utils

    x = np.asarray(x, dtype=np.float32)
    B = x.shape[0]
    weight = np.asarray(weight, dtype=np.float32)
    bias = np.asarray(bias, dtype=np.float32)

    consts = _host_consts(
        np.asarray(mean_norm_weight, dtype=np.float32),
        np.asarray(var_norm_weight, dtype=np.float32),
        np.asarray(pos_w, dtype=np.float32),
        np.asarray(pos_b, dtype=np.float32))

    key = "v5"
    if key not in _PROGRAM_CACHE:
        _PROGRAM_CACHE[key] = _build_program(consts)
    nc = _PROGRAM_CACHE[key]

    in_maps = _make_in_maps(dict(
        x=x, mean_norm_weight=mean_norm_weight,
        var_norm_weight=var_norm_weight, pos_w=pos_w, pos_b=pos_b))

    res = bass_utils.run_bass_kernel_spmd(nc, in_maps,
                                          core_ids=list(range(NCORES)))
    out = np.concatenate(
        [np.asarray(res.results[c]["out"]).reshape(BP, T, C)
         for c in range(NCORES)], axis=0)
    assert out.shape == (B, T, C)
    out = out.astype(np.float32)

    if np.any(weight != 1.0):
        out = out * weight.reshape(1, 1, C)
    if np.any(bias != 0.0):
        out = out + bias.reshape(1, 1, C)
    return out
